# revision 15
# baseline (speedup 1.0000x reference)
"""Trainium2 Bass kernel for Linformer-style sparse attention.

Problem shapes (hardcoded): B=4, S=4096, D=1024, H=16, HD=64, LK=256.

Sharding (8 cores): core c -> (batch b = c//2, sequence half = c%2).
Each core:
  - computes Q/K/V for its 2048 rows (all heads),
  - computes partial [Kp^T; Vp^T] = (K|V)^T @ E^T over its rows,
  - pair AllReduce ([0,1],[2,3],[4,5],[6,7]) completes Kp/Vp,
  - attention (softmax over LK=256) + output projection for its own rows,
  - writes its [2048, 1024] slice of the output directly (no final collective).

All inputs are pre-transposed and pre-cast to bf16 on the host in exact
SBUF layouts, so the device does no transposes or cast round-trips:
  XT [128, 8, 2048]       X^T with d=dc*128+p
  EA/EB [16, 128, 8, 256] E^T per seq-chunk for head groups 0-7 / 8-15
  WK/WV/WQ/WO [128, 8, 1024] with d_in = o*128+p; WQ pre-scaled by 1/sqrt(HD)

Schedule: the K/V projection runs in two per-head-group passes, each fused
with its Kp/Vp partial accumulation (PSUM accumulators across the 16
seq-chunks, E^T streaming in 0.5 MiB chunks).  AllReduce A kicks at ~50%
of the KV work and hides under pass B; AllReduce B hides under the Q
projection.  Readback DMAs ride the ACT HWDGE ring (the GpSimd SWDGE
desc-gen at ~0.6us each would serialize).  The attention loop is
software-pipelined one iteration ahead so the PE FIFO never waits on exp.
"""

import sys

sys.path.insert(0, "/opt/trn_rl_repo")

from contextlib import ExitStack

import numpy as np
import ml_dtypes

from concourse import bacc, bass_isa, bass_utils, mybir, tile
from concourse.masks import make_identity

B, S, D = 4, 4096, 1024
H, HD, LK = 16, 64, 256
SL = S // 2            # local sequence rows per core
P = 128
NSC = SL // P          # 16 s-chunks of 128
NDC = D // P           # 8 d-chunks of 128
NSN = SL // 512        # 4 s-blocks of 512
f32 = mybir.dt.float32
bf16 = mybir.dt.bfloat16
PAIRS = [[0, 1], [2, 3], [4, 5], [6, 7]]
BF16 = ml_dtypes.bfloat16


def _build(include_biases: bool, debug: bool = False):
    nc = bacc.Bacc("TRN2", target_bir_lowering=False, num_devices=8)

    XT_e = nc.declare_dram_parameter("XT", [4, P, NDC, 512], bf16, isOutput=False)
    EA_e = nc.declare_dram_parameter("EA", [NSC, P, 8, LK], bf16, isOutput=False)
    EB_e = nc.declare_dram_parameter("EB", [NSC, P, 8, LK], bf16, isOutput=False)
    WK_e = nc.declare_dram_parameter("WK", [2, P, NDC, 512], bf16, isOutput=False)
    WV_e = nc.declare_dram_parameter("WV", [2, P, NDC, 512], bf16, isOutput=False)
    WQ_e = nc.declare_dram_parameter("WQ", [P, NDC, D], bf16, isOutput=False)
    WO_e = nc.declare_dram_parameter("WO", [P, NDC, D], bf16, isOutput=False)
    BQ_e = nc.declare_dram_parameter("BQ", [P, NDC], f32, isOutput=False)
    MS_e = nc.declare_dram_parameter("MS", [P, NSC], f32, isOutput=False)
    BO_e = nc.declare_dram_parameter("BO", [D], f32, isOutput=False)
    if include_biases:
        BKV_e = nc.declare_dram_parameter("BKV", [2, D], f32, isOutput=False)
    out_e = nc.declare_dram_parameter("out", [SL, D], f32, isOutput=True)
    if debug:
        dbg_cca = nc.declare_dram_parameter("dbg_cca", [8, P, LK], f32, isOutput=True)
        dbg_ccb = nc.declare_dram_parameter("dbg_ccb", [8, P, LK], f32, isOutput=True)
        dbg_qT = nc.declare_dram_parameter("dbg_qT", [P, NDC, SL], f32, isOutput=True)
        dbg_kpT = nc.declare_dram_parameter("dbg_kpT", [P, H // 2, LK], f32, isOutput=True)
        dbg_vp = nc.declare_dram_parameter("dbg_vp", [P, H, 2, HD], f32, isOutput=True)
        dbg_xoT = nc.declare_dram_parameter("dbg_xoT", [P, NDC, SL], f32, isOutput=True)

    # AllReduce bounce (bf16): per head [KpT ; VpT] stacked [128, 256]
    cc_in_a = nc.dram_tensor("cc_in_a", [8, P, LK], bf16, kind="Internal")
    cc_out_a = nc.dram_tensor("cc_out_a", [8, P, LK], bf16, kind="Internal")
    cc_in_b = nc.dram_tensor("cc_in_b", [8, P, LK], bf16, kind="Internal")
    cc_out_b = nc.dram_tensor("cc_out_b", [8, P, LK], bf16, kind="Internal")

    with tile.TileContext(nc) as tc:
        ctx = ExitStack()
        with ctx:
            const_pool = ctx.enter_context(tc.tile_pool(name="consts", bufs=1))

            # ---------------- constants ----------------
            m_sb = const_pool.tile([P, NSC], f32, name="m_sb")
            nc.scalar.dma_start(m_sb[:], MS_e.ap())
            bq_sb = const_pool.tile([P, NDC], f32, name="bq_sb")
            nc.scalar.dma_start(bq_sb[:], BQ_e.ap())
            bo_bc = const_pool.tile([P, D], f32, name="bo_bc")
            nc.gpsimd.dma_start(out=bo_bc[:], in_=BO_e.ap()[None, :].to_broadcast((P, D)))
            if include_biases:
                bkv_bc = const_pool.tile([P, 2, D], f32, name="bkv_bc")
                nc.gpsimd.dma_start(
                    out=bkv_bc[:, 0, :], in_=BKV_e.ap()[0][None, :].to_broadcast((P, D))
                )
                nc.gpsimd.dma_start(
                    out=bkv_bc[:, 1, :], in_=BKV_e.ap()[1][None, :].to_broadcast((P, D))
                )
            id_sb = const_pool.tile([P, P], bf16, name="id_sb")
            make_identity(nc, id_sb[:])
            ones_sb = const_pool.tile([P, HD], bf16, name="ones_sb")
            nc.vector.memset(ones_sb[:], 1.0)

            # ------------- persistent tiles (left stack; frees must be LIFO) ----
            kpT, free_kpT = tc.tile([P, H // 2, LK], bf16, name="kpT")
            vp_sb2, free_vp = tc.tile([P, H, 2, HD], bf16, name="vp_sb2")
            qT, free_qT = tc.tile([P, NDC, SL], bf16, name="qT")
            wq_sb, free_wq = tc.tile([P, NDC, D], bf16, name="wq_sb")
            xt_sb, free_xt = tc.tile([P, NDC, NSN, 512], bf16, name="xt_sb")
            wk_sb, free_wk = tc.tile([P, NDC, 2, 512], bf16, name="wk_sb")
            wv_sb, free_wv = tc.tile([P, NDC, 2, 512], bf16, name="wv_sb")

            # prologue loads: the scalar HWDGE ring comes up several us before
            # the sync ring, so the data the first matmuls need rides it; the
            # rest streams on sync during pass A (xt chunk q feeds sc>=4q)
            def xt_chunk(q, eng):
                eng.dma_start(xt_sb[:, :, q, :], XT_e[q])

            def w_half(w_sb, W_e, half, eng):
                eng.dma_start(w_sb[:, :, half, :], W_e[half])

            w_half(wk_sb, WK_e, 0, nc.sync)
            xt_chunk(0, nc.sync)
            w_half(wv_sb, WV_e, 0, nc.sync)

            # transient pools (right stack)
            kp_scope = ExitStack()
            kp_pool = kp_scope.enter_context(
                tc.tile_pool(name="kp_sb", bufs=4, side="right")
            )
            e_scope = ExitStack()
            e_pool = e_scope.enter_context(
                tc.tile_pool(name="e_pool", bufs=3, side="right")
            )
            kvs_scope = ExitStack()
            kvs_pool = kvs_scope.enter_context(
                tc.tile_pool(name="kvs", bufs=3, side="right")
            )

            ps_scope = ExitStack()
            kv_ps = ps_scope.enter_context(
                tc.tile_pool(name="kv_ps", bufs=4, space="PSUM")
            )

            def kv_pass(E_param, hi, kp_acc, extra=None):
                # K/V projection for one head group (8 heads = weight-column
                # half hi) fused with the Kp/Vp partial accumulation.
                for sc in range(NSC):
                    eT = e_pool.tile([P, 8, LK], bf16, name="eT")
                    nc.sync.dma_start(eT[:], E_param[sc])
                    if extra is not None and sc in extra:
                        extra[sc]()
                    kvs = kvs_pool.tile([P, 8, 2, HD], bf16, name="kvs")
                    for t, w_sb in ((0, wk_sb), (1, wv_sb)):
                        ps = kv_ps.tile([P, 512], f32, name="ps_kv", tag="mm512")
                        for dc in range(NDC):
                            nc.tensor.matmul(
                                ps[:],
                                xt_sb[:, dc, sc // 4, (sc % 4) * P : (sc % 4 + 1) * P],
                                w_sb[:, dc, hi, :],
                                start=(dc == 0),
                                stop=(dc == NDC - 1),
                            )
                        if include_biases:
                            nc.vector.tensor_tensor(
                                out=ps[:],
                                in0=ps[:],
                                in1=bkv_bc[:, t, hi * 512 : (hi + 1) * 512],
                                op=mybir.AluOpType.add,
                            )
                        nc.vector.tensor_scalar(
                            out=kvs[:, :, t, :],
                            in0=ps[:],
                            scalar1=m_sb[:, sc : sc + 1],
                            scalar2=None,
                            op0=mybir.AluOpType.mult,
                        )
                    for hp in range(4):
                        for i in range(2):
                            h = 2 * hp + i
                            nc.tensor.matmul(
                                kp_acc[hp][:, i, :],
                                kvs[:, h, :, :],
                                eT[:, h, :],
                                start=(sc == 0 and i == 0),
                                stop=(sc == NSC - 1 and i == 1),
                                skip_group_check=True,
                            )

            def ship_partials(kp_acc, cc_in):
                for hp in range(4):
                    kp_sb = kp_pool.tile([P, 2, LK], bf16, name="kp_sb")
                    nc.vector.tensor_copy(kp_sb[:], kp_acc[hp][:])
                    for i in range(2):
                        nc.scalar.dma_start(out=cc_in[2 * hp + i], in_=kp_sb[:, i, :])

            # ---------------- pass A: heads 0-7 ----------------
            psA_scope = ExitStack()
            kpA_ps = psA_scope.enter_context(
                tc.tile_pool(name="kpA_ps", bufs=1, space="PSUM")
            )
            kpA = [kpA_ps.tile([P, 2, LK], f32, name=f"kpA{i}") for i in range(4)]
            kv_pass(
                EA_e, 0, kpA,
                extra={
                    1: lambda: xt_chunk(1, nc.sync),
                    2: lambda: xt_chunk(2, nc.sync),
                    3: lambda: xt_chunk(3, nc.sync),
                    6: lambda: w_half(wk_sb, WK_e, 1, nc.sync),
                    8: lambda: w_half(wv_sb, WV_e, 1, nc.sync),
                    10: lambda: nc.sync.dma_start(wq_sb[:], WQ_e.ap()),
                },
            )
            ship_partials(kpA, cc_in_a)
            psA_scope.close()
            nc.gpsimd.collective_compute(
                "AllReduce",
                mybir.AluOpType.add,
                replica_groups=PAIRS,
                ins=[cc_in_a[:].opt()],
                outs=[cc_out_a[:].opt()],
            )
            # readback A rides the ACT HWDGE ring while pass B computes
            vpTA_scope = ExitStack()
            vpTA_pool = vpTA_scope.enter_context(
                tc.tile_pool(name="vpTA", bufs=8, side="right")
            )
            vpTA = []
            for hl in range(8):
                par = (hl % 2) * 64
                nc.scalar.dma_start(
                    out=kpT[par : par + 64, hl // 2, :], in_=cc_out_a[hl, 0:64, :]
                )
                vpT_sb = vpTA_pool.tile([64, 2, P], bf16, name="vpT_sb")
                nc.scalar.dma_start(out=vpT_sb[:], in_=cc_out_a[hl, 64:128, :])
                vpTA.append((hl, vpT_sb))

            # ---------------- pass B: heads 8-15 ----------------
            psB_scope = ExitStack()
            kpB_ps = psB_scope.enter_context(
                tc.tile_pool(name="kpB_ps", bufs=1, space="PSUM")
            )
            kpB = [kpB_ps.tile([P, 2, LK], f32, name=f"kpB{i}") for i in range(4)]
            kv_pass(EB_e, 1, kpB)
            ship_partials(kpB, cc_in_b)
            psB_scope.close()
            ps_scope.close()
            nc.gpsimd.collective_compute(
                "AllReduce",
                mybir.AluOpType.add,
                replica_groups=PAIRS,
                ins=[cc_in_b[:].opt()],
                outs=[cc_out_b[:].opt()],
            )

            # vp transposes for group A (PE work, fits between Q groups)
            psT_scope = ExitStack()
            tp_ps_pool = psT_scope.enter_context(
                tc.tile_pool(name="tp_ps", bufs=2, space="PSUM")
            )
            q_ps_pool = psT_scope.enter_context(
                tc.tile_pool(name="q_ps", bufs=4, space="PSUM")
            )

            def vp_transpose(h, vpT_sb):
                for c in range(2):
                    tp_ps = tp_ps_pool.tile([P, HD], bf16, name="tp_ps")
                    nc.tensor.transpose(tp_ps[:], vpT_sb[:, c, :], id_sb[0:64, 0:64])
                    nc.vector.tensor_copy(vp_sb2[:, h, c, :], tp_ps[:])

            for hl, vpT_sb in vpTA:
                vp_transpose(hl, vpT_sb)

            # readback B DMAs (ACT ring, gated on AllReduce B)
            vpTB = []
            for hl in range(8):
                h = 8 + hl
                par = (h % 2) * 64
                nc.scalar.dma_start(
                    out=kpT[par : par + 64, h // 2, :], in_=cc_out_b[hl, 0:64, :]
                )
                vpT_sb = vpTA_pool.tile([64, 2, P], bf16, name="vpT_sb")
                nc.scalar.dma_start(out=vpT_sb[:], in_=cc_out_b[hl, 64:128, :])
                vpTB.append((h, vpT_sb))

            # ---------------- Q projection (covers AllReduce B) ----------------
            for mc in range(NDC):
                for sn in range(NSN):
                    ps = q_ps_pool.tile([P, 512], f32, name="psq")
                    for dc in range(NDC):
                        nc.tensor.matmul(
                            ps[:],
                            wq_sb[:, dc, mc * P : (mc + 1) * P],
                            xt_sb[:, dc, sn, :],
                            start=(dc == 0),
                            stop=(dc == NDC - 1),
                        )
                    nc.vector.tensor_scalar(
                        out=qT[:, mc, sn * 512 : (sn + 1) * 512],
                        in0=ps[:],
                        scalar1=bq_sb[:, mc : mc + 1],
                        scalar2=None,
                        op0=mybir.AluOpType.add,
                    )

            for h, vpT_sb in vpTB:
                vp_transpose(h, vpT_sb)
            psT_scope.close()
            vpTA_scope.close()
            kvs_scope.close()
            e_scope.close()
            kp_scope.close()
            free_wv()
            free_wk()
            free_xt()

            # ---------------- attention + inline output projection --------------
            wo_sb, free_wo = tc.tile([P, NDC, D], bf16, name="wo_sb")
            nc.sync.dma_start(wo_sb[:], WO_e.ap())
            xoT, free_xoT = tc.tile([P, NDC, SL], bf16, name="xoT")

            with (
                tc.tile_pool(name="at_pool", bufs=4, side="right") as at_pool,
                tc.tile_pool(name="rbc_pool", bufs=2, side="right") as rbc_pool,
                tc.tile_pool(name="osb_pool", bufs=3, side="right") as osb_pool,
                tc.tile_pool(name="ps_dot", bufs=4, space="PSUM") as ps_dot,
                tc.tile_pool(name="ps_xoden", bufs=2, space="PSUM") as ps_xoden,
                tc.tile_pool(name="ps_out", bufs=2, space="PSUM") as ps_out,
            ):
                def attn_dot(sn, j):
                    # heads (2j, 2j+1): even head on partitions 0-63, odd on 64-127
                    ssl = slice(sn * 512, (sn + 1) * 512)
                    ats = []
                    dps = {}
                    for kc in range(2):
                        for pi, par in ((0, 0), (1, 64)):
                            d = ps_dot.tile([P, 512], f32, name="dps")
                            nc.tensor.matmul(
                                d[:],
                                kpT[par : par + 64, j, kc * P : (kc + 1) * P],
                                qT[par : par + 64, j, ssl],
                                start=True,
                                stop=True,
                                tile_position=(par, 0),
                            )
                            dps[(kc, pi)] = d
                    for pi in range(2):
                        at = at_pool.tile([P, 2, 512], bf16, name="at")
                        for kc in range(2):
                            nc.scalar.activation(
                                out=at[:, kc, :],
                                in_=dps[(kc, pi)][:],
                                func=mybir.ActivationFunctionType.Exp,
                            )
                        ats.append(at)
                    return ats

                def attn_pv(sn, j, ats):
                    ssl = slice(sn * 512, (sn + 1) * 512)
                    xo_ps = ps_xoden.tile([P, 512], f32, name="xo_ps", tag="xoden")
                    den_ps = ps_xoden.tile([P, 512], f32, name="den_ps", tag="xoden")
                    for kc in range(2):
                        for pi, par in ((0, 0), (1, 64)):
                            h = 2 * j + pi
                            nc.tensor.matmul(
                                xo_ps[par : par + 64, :],
                                vp_sb2[:, h, kc, :],
                                ats[pi][:, kc, :],
                                start=(kc == 0),
                                stop=(kc == 1),
                                skip_group_check=True,
                            )
                            nc.tensor.matmul(
                                den_ps[par : par + 64, :],
                                ones_sb[:],
                                ats[pi][:, kc, :],
                                start=(kc == 0),
                                stop=(kc == 1),
                                skip_group_check=True,
                            )
                    rbc = rbc_pool.tile([P, 512], f32, name="rbc")
                    nc.vector.reciprocal_approx_fast(out=rbc[:], in_=den_ps[:])
                    nc.vector.tensor_tensor(
                        out=xoT[:, j, ssl],
                        in0=xo_ps[:],
                        in1=rbc[:],
                        op=mybir.AluOpType.mult,
                    )

                def outproj(sn):
                    for si in range(4):
                        sc = sn * 4 + si
                        osb = osb_pool.tile([P, D], f32, name="osb")
                        for half in range(2):
                            ps = ps_out.tile([P, 512], f32, name="ps_o")
                            for c in range(NDC):
                                nc.tensor.matmul(
                                    ps[:],
                                    xoT[:, c, sc * P : (sc + 1) * P],
                                    wo_sb[:, c, half * 512 : (half + 1) * 512],
                                    start=(c == 0),
                                    stop=(c == NDC - 1),
                                )
                            nc.vector.tensor_tensor(
                                out=osb[:, half * 512 : (half + 1) * 512],
                                in0=ps[:],
                                in1=bo_bc[:, half * 512 : (half + 1) * 512],
                                op=mybir.AluOpType.add,
                            )
                        eng = nc.sync if sc % 2 == 0 else nc.scalar
                        eng.dma_start(
                            out=out_e[sc * P : (sc + 1) * P, :], in_=osb[:]
                        )

                # software-pipelined by one iteration: dot(i+1) is emitted before
                # pv(i) so the PE FIFO never stalls on the exp of the current tile
                iters = [(sn, j) for sn in range(NSN) for j in range(H // 2)]
                pending = None
                for sn, j in iters:
                    ats = attn_dot(sn, j)
                    if pending is not None:
                        psn, pj, pats = pending
                        attn_pv(psn, pj, pats)
                        if pj == H // 2 - 1:
                            outproj(psn)
                    pending = (sn, j, ats)
                psn, pj, pats = pending
                attn_pv(psn, pj, pats)
                outproj(psn)

            if debug:
                nc.gpsimd.dma_start(out=dbg_cca[:], in_=cc_in_a[:])
                nc.gpsimd.dma_start(out=dbg_ccb[:], in_=cc_in_b[:])
                nc.gpsimd.dma_start(out=dbg_qT[:], in_=qT[:])
                nc.gpsimd.dma_start(out=dbg_kpT[:], in_=kpT[:])
                nc.gpsimd.dma_start(out=dbg_vp[:], in_=vp_sb2[:])
                nc.gpsimd.dma_start(out=dbg_xoT[:], in_=xoT[:])
            free_xoT()
            free_wo()
            free_wq()
            free_qT()
            free_vp()
            free_kpT()

    nc.compile()
    return nc


_cache = {}


def _get_nc(include_biases: bool, debug: bool = False):
    key = (include_biases, debug)
    if key not in _cache:
        _cache[key] = _build(include_biases, debug)
    return _cache[key]


def prepare_in_maps(inputs):
    X = np.asarray(inputs["X"], np.float32)
    mask = np.asarray(inputs["mask"], np.float32)
    E = np.asarray(inputs["E"], np.float32)
    Ws = {k: np.asarray(inputs[k], np.float32) for k in ("Wq", "Wk", "Wv", "Wo")}
    bs = {k: np.asarray(inputs[k], np.float32) for k in ("bq", "bk", "bv", "bo")}

    include_biases = bool(np.any(bs["bk"]) or np.any(bs["bv"]))

    def wprep(w):
        return np.ascontiguousarray(
            w.reshape(NDC, P, D).transpose(1, 0, 2)
        ).astype(BF16)

    def wprep_h(w):
        # [2, 128, 8, 512]: column halves leading so each half loads contiguously
        return np.ascontiguousarray(
            w.reshape(NDC, P, 2, 512).transpose(2, 1, 0, 3)
        ).astype(BF16)

    WK = wprep_h(Ws["Wk"])
    WV = wprep_h(Ws["Wv"])
    WQ = wprep(Ws["Wq"] * 0.125)
    WO = wprep(Ws["Wo"])
    BQ = np.ascontiguousarray((bs["bq"] * 0.125).reshape(NDC, P).T)
    BO = bs["bo"]
    BKV = np.stack([bs["bk"], bs["bv"]])

    # E^T once: [S, H, LK]
    ET = np.ascontiguousarray(E.transpose(2, 0, 1))
    E_half = {}
    for half in range(2):
        sl = slice(half * SL, (half + 1) * SL)
        Eh = ET[sl].astype(BF16)  # [SL, H, LK]
        EA = np.ascontiguousarray(Eh[:, 0:8, :]).reshape(NSC, P, 8, LK)
        EB = np.ascontiguousarray(Eh[:, 8:16, :]).reshape(NSC, P, 8, LK)
        E_half[half] = (EA, EB)

    in_maps = []
    for c in range(8):
        b, half = c // 2, c % 2
        sl = slice(half * SL, (half + 1) * SL)
        XT = np.ascontiguousarray(
            X[b, sl, :].T.reshape(NDC, P, 4, 512).transpose(2, 1, 0, 3)
        ).astype(BF16)
        MS = np.ascontiguousarray(mask[b, sl].reshape(NSC, P).T)
        EA, EB = E_half[half]
        m = {
            "XT": XT, "EA": EA, "EB": EB,
            "WK": WK, "WV": WV, "WQ": WQ, "WO": WO,
            "BQ": BQ, "MS": MS, "BO": BO,
        }
        if include_biases:
            m["BKV"] = BKV
        in_maps.append(m)
    return include_biases, in_maps


def kernel(**inputs) -> np.ndarray:
    include_biases, in_maps = prepare_in_maps(inputs)
    nc = _get_nc(include_biases)
    res = bass_utils.run_bass_kernel_spmd(nc, in_maps, core_ids=list(range(8)))
    out = np.empty((B, S, D), np.float32)
    for c in range(8):
        b, half = c // 2, c % 2
        out[b, half * SL : (half + 1) * SL, :] = res.results[c]["out"]
    return out


# revision 16
# speedup vs baseline: 1.1659x; 1.1659x over previous
"""Trainium2 Bass kernel for Linformer-style sparse attention.

Problem shapes (hardcoded): B=4, S=4096, D=1024, H=16, HD=64, LK=256.

Sharding (8 cores): core c -> (batch b = c//2, sequence half = c%2).
Each core:
  - computes Q/K/V for its 2048 rows (all heads),
  - computes partial [Kp^T; Vp^T] = (K|V)^T @ E^T over its rows,
  - pair AllReduce ([0,1],[2,3],[4,5],[6,7]) completes Kp/Vp,
  - attention (softmax over LK=256) + output projection for its own rows,
  - writes its [2048, 1024] slice of the output directly (no final collective).

All inputs are pre-transposed and pre-cast to bf16 on the host in exact
SBUF layouts, so the device does no transposes or cast round-trips:
  XT [128, 8, 2048]       X^T with d=dc*128+p
  EA/EB [16, 128, 8, 256] E^T per seq-chunk for head groups 0-7 / 8-15
  WK/WV/WQ/WO [128, 8, 1024] with d_in = o*128+p; WQ pre-scaled by 1/sqrt(HD)

Schedule: the K/V projection runs in two per-head-group passes, each fused
with its Kp/Vp partial accumulation (PSUM accumulators across the 16
seq-chunks, E^T streaming in 0.5 MiB chunks).  AllReduce A kicks at ~50%
of the KV work and hides under pass B; AllReduce B hides under the Q
projection.  Readback DMAs ride the ACT HWDGE ring (the GpSimd SWDGE
desc-gen at ~0.6us each would serialize).  The attention loop is
software-pipelined one iteration ahead so the PE FIFO never waits on exp.
"""

import sys

sys.path.insert(0, "/opt/trn_rl_repo")

from contextlib import ExitStack

import numpy as np
import ml_dtypes

from concourse import bacc, bass_isa, bass_utils, mybir, tile
from concourse.masks import make_identity

B, S, D = 4, 4096, 1024
H, HD, LK = 16, 64, 256
SL = S // 2            # local sequence rows per core
P = 128
NSC = SL // P          # 16 s-chunks of 128
NDC = D // P           # 8 d-chunks of 128
NSN = SL // 512        # 4 s-blocks of 512
f32 = mybir.dt.float32
bf16 = mybir.dt.bfloat16
PAIRS = [[0, 1], [2, 3], [4, 5], [6, 7]]
BF16 = ml_dtypes.bfloat16


def _build(include_biases: bool, debug: bool = False):
    nc = bacc.Bacc("TRN2", target_bir_lowering=False, num_devices=8)

    XT_e = nc.declare_dram_parameter("XT", [4, P, NDC, 512], bf16, isOutput=False)
    EA_e = nc.declare_dram_parameter("EA", [NSC, P, 8, LK], bf16, isOutput=False)
    EB_e = nc.declare_dram_parameter("EB", [NSC, P, 8, LK], bf16, isOutput=False)
    WK_e = nc.declare_dram_parameter("WK", [2, P, NDC, 512], bf16, isOutput=False)
    WV_e = nc.declare_dram_parameter("WV", [2, P, NDC, 512], bf16, isOutput=False)
    WQ_e = nc.declare_dram_parameter("WQ", [P, NDC, D], bf16, isOutput=False)
    WO_e = nc.declare_dram_parameter("WO", [P, NDC, D], bf16, isOutput=False)
    BQ_e = nc.declare_dram_parameter("BQ", [P, NDC], f32, isOutput=False)
    MS_e = nc.declare_dram_parameter("MS", [P, NSC], f32, isOutput=False)
    BO_e = nc.declare_dram_parameter("BO", [D], f32, isOutput=False)
    if include_biases:
        BKV_e = nc.declare_dram_parameter("BKV", [2, D], f32, isOutput=False)
    out_e = nc.declare_dram_parameter("out", [SL, D], f32, isOutput=True)
    if debug:
        dbg_cca = nc.declare_dram_parameter("dbg_cca", [8, P, LK], f32, isOutput=True)
        dbg_ccb = nc.declare_dram_parameter("dbg_ccb", [8, P, LK], f32, isOutput=True)
        dbg_qT = nc.declare_dram_parameter("dbg_qT", [P, NDC, SL], f32, isOutput=True)
        dbg_kpT = nc.declare_dram_parameter("dbg_kpT", [P, H // 2, LK], f32, isOutput=True)
        dbg_vp = nc.declare_dram_parameter("dbg_vp", [P, H, 2, HD], f32, isOutput=True)
        dbg_xoT = nc.declare_dram_parameter("dbg_xoT", [P, NDC, SL], f32, isOutput=True)

    # AllReduce bounce (bf16): per head [KpT ; VpT] stacked [128, 256]
    cc_in_a = nc.dram_tensor("cc_in_a", [8, P, LK], bf16, kind="Internal")
    cc_out_a = nc.dram_tensor("cc_out_a", [8, P, LK], bf16, kind="Internal")
    cc_in_b = nc.dram_tensor("cc_in_b", [8, P, LK], bf16, kind="Internal")
    cc_out_b = nc.dram_tensor("cc_out_b", [8, P, LK], bf16, kind="Internal")

    with tile.TileContext(nc) as tc:
        ctx = ExitStack()
        with ctx:
            const_pool = ctx.enter_context(tc.tile_pool(name="consts", bufs=1))

            # ---------------- constants ----------------
            m_sb = const_pool.tile([P, NSC], f32, name="m_sb")
            nc.scalar.dma_start(m_sb[:], MS_e.ap())
            bq_sb = const_pool.tile([P, NDC], f32, name="bq_sb")
            nc.scalar.dma_start(bq_sb[:], BQ_e.ap())
            bo_bc = const_pool.tile([P, D], f32, name="bo_bc")
            nc.gpsimd.dma_start(out=bo_bc[:], in_=BO_e.ap()[None, :].to_broadcast((P, D)))
            if include_biases:
                bkv_bc = const_pool.tile([P, 2, D], f32, name="bkv_bc")
                nc.gpsimd.dma_start(
                    out=bkv_bc[:, 0, :], in_=BKV_e.ap()[0][None, :].to_broadcast((P, D))
                )
                nc.gpsimd.dma_start(
                    out=bkv_bc[:, 1, :], in_=BKV_e.ap()[1][None, :].to_broadcast((P, D))
                )
            id_sb = const_pool.tile([P, P], bf16, name="id_sb")
            make_identity(nc, id_sb[:])
            ones_sb = const_pool.tile([P, HD], bf16, name="ones_sb")
            nc.vector.memset(ones_sb[:], 1.0)

            # ------------- persistent tiles (left stack; frees must be LIFO) ----
            kpT, free_kpT = tc.tile([P, H // 2, LK], bf16, name="kpT")
            vp_sb2, free_vp = tc.tile([P, H, 2, HD], bf16, name="vp_sb2")
            qT, free_qT = tc.tile([P, NDC, SL], bf16, name="qT")
            wq_sb, free_wq = tc.tile([P, NDC, D], bf16, name="wq_sb")
            xt_sb, free_xt = tc.tile([P, NDC, NSN, 512], bf16, name="xt_sb")
            wk_sb, free_wk = tc.tile([P, NDC, 2, 512], bf16, name="wk_sb")
            wv_sb, free_wv = tc.tile([P, NDC, 2, 512], bf16, name="wv_sb")

            # prologue loads: the scalar HWDGE ring comes up several us before
            # the sync ring, so the data the first matmuls need rides it; the
            # rest streams on sync during pass A (xt chunk q feeds sc>=4q)
            def xt_chunk(q, eng):
                eng.dma_start(xt_sb[:, :, q, :], XT_e[q])

            def w_half(w_sb, W_e, half, eng):
                eng.dma_start(w_sb[:, :, half, :], W_e[half])

            # first-needed slices ride first, split per-dc so the first
            # matmuls (and the HAM warm-up) start as early as possible
            for dc in range(NDC):
                nc.sync.dma_start(wk_sb[:, dc, 0, :], WK_e[0, :, dc, :])
                nc.sync.dma_start(xt_sb[:, dc, 0, :], XT_e[0, :, dc, :])
            w_half(wv_sb, WV_e, 0, nc.sync)

            # transient pools (right stack)
            kp_scope = ExitStack()
            kp_pool = kp_scope.enter_context(
                tc.tile_pool(name="kp_sb", bufs=4, side="right")
            )
            e_scope = ExitStack()
            e_pool = e_scope.enter_context(
                tc.tile_pool(name="e_pool", bufs=3, side="right")
            )
            kvs_scope = ExitStack()
            kvs_pool = kvs_scope.enter_context(
                tc.tile_pool(name="kvs", bufs=4, side="right")
            )

            ps_scope = ExitStack()
            kv_ps = ps_scope.enter_context(
                tc.tile_pool(name="kv_ps", bufs=4, space="PSUM")
            )

            def kv_pass(E_param, hi, kp_acc, extra=None):
                # K/V projection for one head group (8 heads = weight-column
                # half hi) fused with the Kp/Vp partial accumulation.
                for sc in range(NSC):
                    eT = e_pool.tile([P, 8, LK], bf16, name="eT")
                    nc.sync.dma_start(eT[:], E_param[sc])
                    if extra is not None and sc in extra:
                        extra[sc]()
                    kvs = kvs_pool.tile([P, 8, 2, HD], bf16, name="kvs")
                    for t, w_sb in ((0, wk_sb), (1, wv_sb)):
                        ps = kv_ps.tile([P, 512], f32, name="ps_kv", tag="mm512")
                        for dc in range(NDC):
                            nc.tensor.matmul(
                                ps[:],
                                xt_sb[:, dc, sc // 4, (sc % 4) * P : (sc % 4 + 1) * P],
                                w_sb[:, dc, hi, :],
                                start=(dc == 0),
                                stop=(dc == NDC - 1),
                            )
                        if include_biases:
                            nc.vector.tensor_tensor(
                                out=ps[:],
                                in0=ps[:],
                                in1=bkv_bc[:, t, hi * 512 : (hi + 1) * 512],
                                op=mybir.AluOpType.add,
                            )
                        nc.vector.tensor_scalar(
                            out=kvs[:, :, t, :],
                            in0=ps[:],
                            scalar1=m_sb[:, sc : sc + 1],
                            scalar2=None,
                            op0=mybir.AluOpType.mult,
                        )
                    for hp in range(4):
                        for i in range(2):
                            h = 2 * hp + i
                            nc.tensor.matmul(
                                kp_acc[hp][:, i, :],
                                kvs[:, h, :, :],
                                eT[:, h, :],
                                start=(sc == 0 and i == 0),
                                stop=(sc == NSC - 1 and i == 1),
                                skip_group_check=True,
                            )

            def ship_partials(kp_acc, cc_in):
                for hp in range(4):
                    kp_sb = kp_pool.tile([P, 2, LK], bf16, name="kp_sb")
                    nc.vector.tensor_copy(kp_sb[:], kp_acc[hp][:])
                    for i in range(2):
                        nc.scalar.dma_start(out=cc_in[2 * hp + i], in_=kp_sb[:, i, :])

            # ---------------- pass A: heads 0-7 ----------------
            psA_scope = ExitStack()
            kpA_ps = psA_scope.enter_context(
                tc.tile_pool(name="kpA_ps", bufs=1, space="PSUM")
            )
            kpA = [kpA_ps.tile([P, 2, LK], f32, name=f"kpA{i}") for i in range(4)]
            kv_pass(
                EA_e, 0, kpA,
                extra={
                    1: lambda: xt_chunk(1, nc.sync),
                    2: lambda: xt_chunk(2, nc.sync),
                    3: lambda: xt_chunk(3, nc.sync),
                    6: lambda: w_half(wk_sb, WK_e, 1, nc.sync),
                    8: lambda: w_half(wv_sb, WV_e, 1, nc.sync),
                    10: lambda: nc.sync.dma_start(wq_sb[:], WQ_e.ap()),
                },
            )
            ship_partials(kpA, cc_in_a)
            psA_scope.close()
            nc.gpsimd.collective_compute(
                "AllReduce",
                mybir.AluOpType.add,
                replica_groups=PAIRS,
                ins=[cc_in_a[:].opt()],
                outs=[cc_out_a[:].opt()],
            )
            # readback A rides the ACT HWDGE ring while pass B computes
            vpTA_scope = ExitStack()
            vpTA_pool = vpTA_scope.enter_context(
                tc.tile_pool(name="vpTA", bufs=8, side="right")
            )
            vpTA = []
            for hl in range(8):
                par = (hl % 2) * 64
                nc.scalar.dma_start(
                    out=kpT[par : par + 64, hl // 2, :], in_=cc_out_a[hl, 0:64, :]
                )
                vpT_sb = vpTA_pool.tile([64, 2, P], bf16, name="vpT_sb")
                nc.scalar.dma_start(out=vpT_sb[:], in_=cc_out_a[hl, 64:128, :])
                vpTA.append((hl, vpT_sb))

            # ---------------- pass B: heads 8-15 ----------------
            psB_scope = ExitStack()
            kpB_ps = psB_scope.enter_context(
                tc.tile_pool(name="kpB_ps", bufs=1, space="PSUM")
            )
            kpB = [kpB_ps.tile([P, 2, LK], f32, name=f"kpB{i}") for i in range(4)]
            kv_pass(EB_e, 1, kpB)
            ship_partials(kpB, cc_in_b)
            psB_scope.close()
            ps_scope.close()
            nc.gpsimd.collective_compute(
                "AllReduce",
                mybir.AluOpType.add,
                replica_groups=PAIRS,
                ins=[cc_in_b[:].opt()],
                outs=[cc_out_b[:].opt()],
            )

            # vp transposes for group A (PE work, fits between Q groups)
            psT_scope = ExitStack()
            tp_ps_pool = psT_scope.enter_context(
                tc.tile_pool(name="tp_ps", bufs=2, space="PSUM")
            )
            q_ps_pool = psT_scope.enter_context(
                tc.tile_pool(name="q_ps", bufs=4, space="PSUM")
            )

            def vp_transpose(h, vpT_sb):
                for c in range(2):
                    tp_ps = tp_ps_pool.tile([P, HD], bf16, name="tp_ps")
                    nc.tensor.transpose(tp_ps[:], vpT_sb[:, c, :], id_sb[0:64, 0:64])
                    nc.vector.tensor_copy(vp_sb2[:, h, c, :], tp_ps[:])

            for hl, vpT_sb in vpTA:
                vp_transpose(hl, vpT_sb)

            # readback B DMAs (ACT ring, gated on AllReduce B)
            vpTB = []
            for hl in range(8):
                h = 8 + hl
                par = (h % 2) * 64
                nc.scalar.dma_start(
                    out=kpT[par : par + 64, h // 2, :], in_=cc_out_b[hl, 0:64, :]
                )
                vpT_sb = vpTA_pool.tile([64, 2, P], bf16, name="vpT_sb")
                nc.scalar.dma_start(out=vpT_sb[:], in_=cc_out_b[hl, 64:128, :])
                vpTB.append((h, vpT_sb))

            # ---------------- Q projection (covers AllReduce B) ----------------
            for mc in range(NDC):
                for sn in range(NSN):
                    ps = q_ps_pool.tile([P, 512], f32, name="psq")
                    for dc in range(NDC):
                        nc.tensor.matmul(
                            ps[:],
                            wq_sb[:, dc, mc * P : (mc + 1) * P],
                            xt_sb[:, dc, sn, :],
                            start=(dc == 0),
                            stop=(dc == NDC - 1),
                        )
                    nc.vector.tensor_scalar(
                        out=qT[:, mc, sn * 512 : (sn + 1) * 512],
                        in0=ps[:],
                        scalar1=bq_sb[:, mc : mc + 1],
                        scalar2=None,
                        op0=mybir.AluOpType.add,
                    )

            for h, vpT_sb in vpTB:
                vp_transpose(h, vpT_sb)
            psT_scope.close()
            vpTA_scope.close()
            kvs_scope.close()
            e_scope.close()
            kp_scope.close()
            free_wv()
            free_wk()
            free_xt()

            # ---------------- attention + inline output projection --------------
            wo_sb, free_wo = tc.tile([P, NDC, D], bf16, name="wo_sb")
            nc.sync.dma_start(wo_sb[:], WO_e.ap())
            xoT, free_xoT = tc.tile([P, NDC, SL], bf16, name="xoT")

            with (
                tc.tile_pool(name="at_pool", bufs=4, side="right") as at_pool,
                tc.tile_pool(name="rbc_pool", bufs=2, side="right") as rbc_pool,
                tc.tile_pool(name="osb_pool", bufs=3, side="right") as osb_pool,
                tc.tile_pool(name="ps_dot", bufs=4, space="PSUM") as ps_dot,
                tc.tile_pool(name="ps_xoden", bufs=2, space="PSUM") as ps_xoden,
                tc.tile_pool(name="ps_out", bufs=2, space="PSUM") as ps_out,
            ):
                def attn_dot(sn, j):
                    # heads (2j, 2j+1): even head on partitions 0-63, odd on 64-127
                    ssl = slice(sn * 512, (sn + 1) * 512)
                    ats = []
                    dps = {}
                    for kc in range(2):
                        for pi, par in ((0, 0), (1, 64)):
                            d = ps_dot.tile([P, 512], f32, name="dps")
                            nc.tensor.matmul(
                                d[:],
                                kpT[par : par + 64, j, kc * P : (kc + 1) * P],
                                qT[par : par + 64, j, ssl],
                                start=True,
                                stop=True,
                                tile_position=(par, 0),
                            )
                            dps[(kc, pi)] = d
                    for pi in range(2):
                        at = at_pool.tile([P, 2, 512], bf16, name="at")
                        for kc in range(2):
                            nc.scalar.activation(
                                out=at[:, kc, :],
                                in_=dps[(kc, pi)][:],
                                func=mybir.ActivationFunctionType.Exp,
                            )
                        ats.append(at)
                    return ats

                def attn_pv(sn, j, ats):
                    ssl = slice(sn * 512, (sn + 1) * 512)
                    xo_ps = ps_xoden.tile([P, 512], f32, name="xo_ps", tag="xoden")
                    den_ps = ps_xoden.tile([P, 512], f32, name="den_ps", tag="xoden")
                    for kc in range(2):
                        for pi, par in ((0, 0), (1, 64)):
                            h = 2 * j + pi
                            nc.tensor.matmul(
                                xo_ps[par : par + 64, :],
                                vp_sb2[:, h, kc, :],
                                ats[pi][:, kc, :],
                                start=(kc == 0),
                                stop=(kc == 1),
                                skip_group_check=True,
                            )
                            nc.tensor.matmul(
                                den_ps[par : par + 64, :],
                                ones_sb[:],
                                ats[pi][:, kc, :],
                                start=(kc == 0),
                                stop=(kc == 1),
                                skip_group_check=True,
                            )
                    rbc = rbc_pool.tile([P, 512], f32, name="rbc")
                    nc.vector.reciprocal_approx_fast(out=rbc[:], in_=den_ps[:])
                    nc.vector.tensor_tensor(
                        out=xoT[:, j, ssl],
                        in0=xo_ps[:],
                        in1=rbc[:],
                        op=mybir.AluOpType.mult,
                    )

                def outproj(sn):
                    for si in range(4):
                        sc = sn * 4 + si
                        osb = osb_pool.tile([P, D], f32, name="osb")
                        for half in range(2):
                            ps = ps_out.tile([P, 512], f32, name="ps_o")
                            for c in range(NDC):
                                nc.tensor.matmul(
                                    ps[:],
                                    xoT[:, c, sc * P : (sc + 1) * P],
                                    wo_sb[:, c, half * 512 : (half + 1) * 512],
                                    start=(c == 0),
                                    stop=(c == NDC - 1),
                                )
                            nc.vector.tensor_tensor(
                                out=osb[:, half * 512 : (half + 1) * 512],
                                in0=ps[:],
                                in1=bo_bc[:, half * 512 : (half + 1) * 512],
                                op=mybir.AluOpType.add,
                            )
                        eng = nc.sync if sc % 2 == 0 else nc.scalar
                        eng.dma_start(
                            out=out_e[sc * P : (sc + 1) * P, :], in_=osb[:]
                        )

                # software-pipelined by one iteration: dot(i+1) is emitted before
                # pv(i) so the PE FIFO never stalls on the exp of the current tile
                iters = [(sn, j) for sn in range(NSN) for j in range(H // 2)]
                pending = None
                for sn, j in iters:
                    ats = attn_dot(sn, j)
                    if pending is not None:
                        psn, pj, pats = pending
                        attn_pv(psn, pj, pats)
                        if pj == H // 2 - 1:
                            outproj(psn)
                    pending = (sn, j, ats)
                psn, pj, pats = pending
                attn_pv(psn, pj, pats)
                outproj(psn)

            if debug:
                nc.gpsimd.dma_start(out=dbg_cca[:], in_=cc_in_a[:])
                nc.gpsimd.dma_start(out=dbg_ccb[:], in_=cc_in_b[:])
                nc.gpsimd.dma_start(out=dbg_qT[:], in_=qT[:])
                nc.gpsimd.dma_start(out=dbg_kpT[:], in_=kpT[:])
                nc.gpsimd.dma_start(out=dbg_vp[:], in_=vp_sb2[:])
                nc.gpsimd.dma_start(out=dbg_xoT[:], in_=xoT[:])
            free_xoT()
            free_wo()
            free_wq()
            free_qT()
            free_vp()
            free_kpT()

    nc.compile()
    return nc


_cache = {}


def _get_nc(include_biases: bool, debug: bool = False):
    key = (include_biases, debug)
    if key not in _cache:
        _cache[key] = _build(include_biases, debug)
    return _cache[key]


def prepare_in_maps(inputs):
    X = np.asarray(inputs["X"], np.float32)
    mask = np.asarray(inputs["mask"], np.float32)
    E = np.asarray(inputs["E"], np.float32)
    Ws = {k: np.asarray(inputs[k], np.float32) for k in ("Wq", "Wk", "Wv", "Wo")}
    bs = {k: np.asarray(inputs[k], np.float32) for k in ("bq", "bk", "bv", "bo")}

    include_biases = bool(np.any(bs["bk"]) or np.any(bs["bv"]))

    def wprep(w):
        return np.ascontiguousarray(
            w.reshape(NDC, P, D).transpose(1, 0, 2)
        ).astype(BF16)

    def wprep_h(w):
        # [2, 128, 8, 512]: column halves leading so each half loads contiguously
        return np.ascontiguousarray(
            w.reshape(NDC, P, 2, 512).transpose(2, 1, 0, 3)
        ).astype(BF16)

    WK = wprep_h(Ws["Wk"])
    WV = wprep_h(Ws["Wv"])
    WQ = wprep(Ws["Wq"] * 0.125)
    WO = wprep(Ws["Wo"])
    BQ = np.ascontiguousarray((bs["bq"] * 0.125).reshape(NDC, P).T)
    BO = bs["bo"]
    BKV = np.stack([bs["bk"], bs["bv"]])

    # E^T once: [S, H, LK]
    ET = np.ascontiguousarray(E.transpose(2, 0, 1))
    E_half = {}
    for half in range(2):
        sl = slice(half * SL, (half + 1) * SL)
        Eh = ET[sl].astype(BF16)  # [SL, H, LK]
        EA = np.ascontiguousarray(Eh[:, 0:8, :]).reshape(NSC, P, 8, LK)
        EB = np.ascontiguousarray(Eh[:, 8:16, :]).reshape(NSC, P, 8, LK)
        E_half[half] = (EA, EB)

    in_maps = []
    for c in range(8):
        b, half = c // 2, c % 2
        sl = slice(half * SL, (half + 1) * SL)
        XT = np.ascontiguousarray(
            X[b, sl, :].T.reshape(NDC, P, 4, 512).transpose(2, 1, 0, 3)
        ).astype(BF16)
        MS = np.ascontiguousarray(mask[b, sl].reshape(NSC, P).T)
        EA, EB = E_half[half]
        m = {
            "XT": XT, "EA": EA, "EB": EB,
            "WK": WK, "WV": WV, "WQ": WQ, "WO": WO,
            "BQ": BQ, "MS": MS, "BO": BO,
        }
        if include_biases:
            m["BKV"] = BKV
        in_maps.append(m)
    return include_biases, in_maps


def kernel(**inputs) -> np.ndarray:
    include_biases, in_maps = prepare_in_maps(inputs)
    nc = _get_nc(include_biases)
    res = bass_utils.run_bass_kernel_spmd(nc, in_maps, core_ids=list(range(8)))
    out = np.empty((B, S, D), np.float32)
    for c in range(8):
        b, half = c // 2, c % 2
        out[b, half * SL : (half + 1) * SL, :] = res.results[c]["out"]
    return out
